# revision 19
# baseline (speedup 1.0000x reference)
"""CTC beam-search decoder kernel for Trainium2 (8 NeuronCores, data-parallel).

Math note: the reference keeps (prefix, score) beams with NO prefix merging and
expands every beam with the SAME per-step log-prob vector, taking a global
top-W each step.  Under jax.lax.top_k's descending sort, beam 0 after step t is
always (previous beam 0) extended by the per-step argmax class, and its score
is the running sum of per-step max log-probs:

    best_seq[b, t]  = argmax_c logits[b, t, c]          (logp is monotone in logits)
    scores[b, 0]    = sum_t (max_c logits[b,t,c] - logsumexp_c logits[b,t,c])

so the whole scan collapses to a per-(b,t)-row argmax + logsumexp, followed by
the reference's blank/repeat collapse of the greedy path.

Per core (batch shard of 8 rows): 64 tiles of [128 rows, 512 classes] are
processed with DVE max8/max_index + ACT exp(accum) for logsumexp; the epilogue
transposes per-row results into [8, T] layout, runs the collapse (cummax
forward-fill of last non-blank class, cumsum compaction indices, GPSIMD
local_scatter) and writes decoded/lengths/scores.  Tail scalar-affine ops ride
the otherwise-idle ACT engine; fold DMAs split across the two HWDGE queues.
"""
import os
import sys

sys.path.insert(0, "/opt/trn_rl_repo")

import numpy as np

import concourse.bacc as bacc
import concourse.mybir as mybir
import concourse.tile as tile
from concourse import masks
from concourse.bass_utils import run_bass_kernel_spmd

B, T, C = 64, 1024, 512
NCORES = 8
KB = B // NCORES          # batch rows per core
P = 128                   # SBUF partitions
NT = KB * T // P          # [P, C] tiles per core
TH = NT // KB             # 128-row chunks per batch row (t = th*128 + p)
W8 = 8                    # max8 width

F32 = mybir.dt.float32
BF16 = mybir.dt.bfloat16
I32 = mybir.dt.int32
I16 = mybir.dt.int16
U32 = mybir.dt.uint32
AF = mybir.ActivationFunctionType
OP = mybir.AluOpType

_CACHE = {}


def _build():
    nc = bacc.Bacc(
        "TRN2",
        target_bir_lowering=False,
        debug=False,
        enable_asserts=False,
        num_devices=1,
    )
    logits = nc.dram_tensor("logits", [KB, T, C], F32, kind="ExternalInput").ap()
    decoded = nc.dram_tensor("decoded", [KB, T], I32, kind="ExternalOutput").ap()
    lengths = nc.dram_tensor("lengths", [KB, 1], I32, kind="ExternalOutput").ap()
    scores = nc.dram_tensor("scores", [KB, 1], F32, kind="ExternalOutput").ap()

    # row r = b*T + t = 128*(2*k2 + two) + p ; double-tile loads
    xv2 = logits.rearrange("b t c -> (b t) c").rearrange(
        "(n two p) c -> n p two c", two=2, p=P)

    with tile.TileContext(nc) as tc:
        with tc.tile_pool(name="xin", bufs=6) as xpool, \
             tc.tile_pool(name="expo", bufs=4) as epool, \
             tc.tile_pool(name="acc", bufs=1) as acc, \
             tc.tile_pool(name="post", bufs=1) as post, \
             tc.tile_pool(name="psumP", bufs=2, space="PSUM") as psum:

            # constants / misc, emitted first so they overlap the loads
            ident = post.tile([P, P], F32, tag="ident")
            masks.make_identity(nc, ident[:])
            ones = post.tile([P, 1], F32, tag="ones")
            nc.vector.memset(ones[:], 1.0)
            # iota: value = t+1 = th*128 + p + 1 per column k=(b,th)
            posf1 = post.tile([P, NT], F32, tag="posf1")
            nc.gpsimd.iota(posf1[:], pattern=[[0, KB], [128, TH]], base=1,
                           channel_multiplier=1,
                           allow_small_or_imprecise_dtypes=True)
            # preload the local_scatter ucode library so the reload's engine
            # drain happens before phase 1, not on the tail
            try:
                from concourse import library_config
                nc.gpsimd.load_library(library_config.local_scatter)
            except Exception:
                pass
            zer = post.tile([KB, T], F32, tag="zer")
            nc.vector.memset(zer[:], 0.0)

            # Per-row accumulators, written tile-column-at-a-time.
            macc = acc.tile([P, NT * W8], F32, tag="macc")     # top8 values
            gacc = acc.tile([P, NT * W8], U32, tag="gacc")     # top8 indices
            sacc = acc.tile([P, NT], F32, tag="sacc")          # sum(exp(x))

            for k2 in range(NT // 2):
                xt = xpool.tile([P, 2 * C], F32, tag="xt")
                nc.sync.dma_start(
                    xt[:].rearrange("p (two c) -> p two c", two=2), xv2[k2])
                for two in range(2):
                    k = 2 * k2 + two
                    xh = xt[:, two * C:(two + 1) * C]
                    m8 = macc[:, k * W8:(k + 1) * W8]
                    nc.vector.max(m8, xh)
                    nc.vector.max_index(gacc[:, k * W8:(k + 1) * W8], m8, xh)
                    sc = epool.tile([P, C], BF16, tag="sc")
                    nc.scalar.activation(sc[:], xh, AF.Exp,
                                         accum_out=sacc[:, k:k + 1])

            # ---- epilogue in [P, NT] layout (row-major (b,t) rows) ----
            mv = macc[:].rearrange("p (k e) -> p k e", e=W8)[:, :, 0]   # [P,NT]
            gv = gacc[:].rearrange("p (k e) -> p k e", e=W8)[:, :, 0]

            ls = post.tile([P, NT], F32, tag="ls")
            nc.scalar.activation(ls[:], sacc[:], AF.Ln)
            ct = post.tile([P, NT], F32, tag="ct")    # contrib = m - ln(sumexp)
            nc.vector.tensor_tensor(ct[:], mv, ls[:], OP.subtract)
            gf = post.tile([P, NT], F32, tag="gf")    # argmax ids as f32
            nc.vector.tensor_copy(gf[:], gv)

            # scores[b] = sum_t ct: PE column-sum then group-of-8 reduce
            cps = psum.tile([1, NT], F32, tag="cps")
            nc.tensor.matmul(cps[:], ones[:, 0:1], ct[:])     # [1, NT] colsums
            s8 = post.tile([1, KB], F32, tag="s8")
            nc.vector.tensor_reduce(
                s8[:], cps[:].rearrange("o (b h) -> o b h", b=KB),
                axis=mybir.AxisListType.X, op=OP.add)
            nc.sync.dma_start(scores, s8[:])

            # enc = nonblank ? (t+1) + g/1024 : 0, built in [P, NT] layout.
            # (exact in f32: (1024*(t+1)+g) < 2^21)
            nb128 = post.tile([P, NT], F32, tag="nb128")
            nc.vector.tensor_scalar(nb128[:], gf[:], 0.0, None, OP.not_equal)
            e1 = post.tile([P, NT], F32, tag="e1")
            nc.vector.scalar_tensor_tensor(e1[:], gf[:], 1.0 / 1024.0, posf1[:],
                                           op0=OP.mult, op1=OP.add)
            enc128 = post.tile([P, NT], F32, tag="enc128")
            nc.vector.tensor_tensor(enc128[:], e1[:], nb128[:], OP.mult)

            gb = post.tile([KB, T], F32, tag="gb")
            encb = post.tile([KB, T], F32, tag="encb")
            for src, dst, tg, dmae in ((gf, gb, "tgf", nc.scalar),
                                       (enc128, encb, "tenc", nc.sync)):
                pt = psum.tile([NT, P], F32, tag="pt")
                nc.tensor.transpose(pt[:], src[:], ident[:])
                sb = post.tile([NT, P], F32, tag=tg)
                nc.vector.tensor_copy(sb[:], pt[:])
                for b in range(KB):
                    dmae.dma_start(dst[b:b + 1, :],
                                   sb[b * TH:(b + 1) * TH, :])

            # ---- collapse of the greedy path, [KB, T] layout ----
            efull = post.tile([KB, T + 1], F32, tag="efull")
            nc.vector.memset(efull[:, 0:1], 0.0)
            nc.vector.tensor_tensor_scan(efull[:, 1:T + 1], encb[:], zer[:],
                                         0.0, OP.max, OP.add)
            pe = efull[:, 0:T]                       # enc of last nonblank < t
            # prev char = frac(pe) * 1024; sentinel 0 -> 0 = blank
            ri = post.tile([KB, T], I32, tag="ri")
            nc.vector.tensor_copy(ri[:], pe)         # trunc (pe >= 0)
            fr = post.tile([KB, T], F32, tag="fr")   # fr = pe - trunc(pe)
            nc.vector.scalar_tensor_tensor(fr[:], ri[:], -1.0, pe,
                                           op0=OP.mult, op1=OP.add)
            pc = post.tile([KB, T], F32, tag="pc")   # on ACT: fr*1024
            nc.scalar.activation(pc[:], fr[:], AF.Copy, scale=1024.0)
            ne2 = post.tile([KB, T], F32, tag="ne2")
            nc.vector.tensor_tensor(ne2[:], gb[:], pc[:], OP.not_equal)
            keep = post.tile([KB, T], F32, tag="keep")   # (gb != 0) * ne2
            nc.vector.scalar_tensor_tensor(keep[:], gb[:], 0.0, ne2[:],
                                           op0=OP.not_equal, op1=OP.mult)

            csum = post.tile([KB, T], F32, tag="csum")
            nc.vector.tensor_tensor_scan(csum[:], keep[:], zer[:], 0.0,
                                         OP.add, OP.add)
            leni = post.tile([KB, 1], I32, tag="leni")
            nc.scalar.activation(leni[:], csum[:, T - 1:T], AF.Copy)
            nc.scalar.dma_start(lengths, leni[:])

            sidf = post.tile([KB, T], F32, tag="sidf")
            nc.vector.tensor_tensor(sidf[:], keep[:], csum[:], OP.mult)
            sid16 = post.tile([16, T], I16, tag="sid16")
            nc.vector.memset(sid16[:], -1)
            nc.scalar.activation(sid16[0:KB, :], sidf[:], AF.Copy, bias=-1.0)
            dat16 = post.tile([16, T], I16, tag="dat16")
            nc.vector.memset(dat16[:], 0)
            nc.scalar.activation(dat16[0:KB, :], gb[:], AF.Copy, bias=1.0)
            dec16 = post.tile([16, T], I16, tag="dec16")
            nc.gpsimd.local_scatter(dec16[:], dat16[:], sid16[:], channels=16,
                                    num_elems=T, num_idxs=T)
            dec32 = post.tile([KB, T], I32, tag="dec32")
            nc.scalar.activation(dec32[:], dec16[0:KB, :], AF.Copy, bias=-1.0)
            nc.sync.dma_start(decoded, dec32[:])

    nc.compile()
    return nc


def _get_nc():
    if "nc" not in _CACHE:
        _CACHE["nc"] = _build()
    return _CACHE["nc"]


def kernel(logits: np.ndarray, _trace: bool = False, _result_box: dict | None = None):
    nc = _get_nc()
    logits = np.ascontiguousarray(logits, dtype=np.float32)
    in_maps = [
        {"logits": logits[c * KB:(c + 1) * KB]} for c in range(NCORES)
    ]
    res = run_bass_kernel_spmd(nc, in_maps, core_ids=list(range(NCORES)),
                               trace=_trace)
    if _result_box is not None:
        _result_box["res"] = res
    decoded = np.concatenate([res.results[c]["decoded"] for c in range(NCORES)], axis=0)
    lengths = np.concatenate([res.results[c]["lengths"] for c in range(NCORES)], axis=0)
    scores = np.concatenate([res.results[c]["scores"] for c in range(NCORES)], axis=0)
    return decoded, lengths.reshape(B), scores.reshape(B)


# revision 21
# speedup vs baseline: 1.0057x; 1.0057x over previous
"""CTC beam-search decoder kernel for Trainium2 (8 NeuronCores, data-parallel).

Math note: the reference keeps (prefix, score) beams with NO prefix merging and
expands every beam with the SAME per-step log-prob vector, taking a global
top-W each step.  Under jax.lax.top_k's descending sort, beam 0 after step t is
always (previous beam 0) extended by the per-step argmax class, and its score
is the running sum of per-step max log-probs:

    best_seq[b, t]  = argmax_c logits[b, t, c]          (logp is monotone in logits)
    scores[b, 0]    = sum_t (max_c logits[b,t,c] - logsumexp_c logits[b,t,c])

so the whole scan collapses to a per-(b,t)-row argmax + logsumexp, followed by
the reference's blank/repeat collapse of the greedy path.

Per core (batch shard of 8 rows): 64 tiles of [128 rows, 512 classes] are
processed with DVE max8/max_index + ACT exp(accum) for logsumexp; the epilogue
transposes per-row results into [8, T] layout, runs the collapse (cummax
forward-fill of last non-blank class, cumsum compaction indices, GPSIMD
local_scatter) and writes decoded/lengths/scores.  Tail scalar-affine ops ride
the otherwise-idle ACT engine; fold DMAs split across the two HWDGE queues.
"""
import os
import sys

sys.path.insert(0, "/opt/trn_rl_repo")

import numpy as np

import concourse.bacc as bacc
import concourse.mybir as mybir
import concourse.tile as tile
from concourse import masks
from concourse.bass_utils import run_bass_kernel_spmd

B, T, C = 64, 1024, 512
NCORES = 8
KB = B // NCORES          # batch rows per core
P = 128                   # SBUF partitions
NT = KB * T // P          # [P, C] tiles per core
TH = NT // KB             # 128-row chunks per batch row (t = th*128 + p)
W8 = 8                    # max8 width

F32 = mybir.dt.float32
BF16 = mybir.dt.bfloat16
I32 = mybir.dt.int32
I16 = mybir.dt.int16
U32 = mybir.dt.uint32
AF = mybir.ActivationFunctionType
OP = mybir.AluOpType

_CACHE = {}


def _build():
    nc = bacc.Bacc(
        "TRN2",
        target_bir_lowering=False,
        debug=False,
        enable_asserts=False,
        num_devices=1,
    )
    logits = nc.dram_tensor("logits", [KB, T, C], F32, kind="ExternalInput").ap()
    decoded = nc.dram_tensor("decoded", [KB, T], I32, kind="ExternalOutput").ap()
    lengths = nc.dram_tensor("lengths", [KB, 1], I32, kind="ExternalOutput").ap()
    scores = nc.dram_tensor("scores", [KB, 1], F32, kind="ExternalOutput").ap()

    # row r = b*T + t = 128*(2*k2 + two) + p ; double-tile loads
    xv2 = logits.rearrange("b t c -> (b t) c").rearrange(
        "(n two p) c -> n p two c", two=2, p=P)

    with tile.TileContext(nc) as tc:
        with tc.tile_pool(name="xin", bufs=6) as xpool, \
             tc.tile_pool(name="expo", bufs=4) as epool, \
             tc.tile_pool(name="acc", bufs=1) as acc, \
             tc.tile_pool(name="post", bufs=1) as post, \
             tc.tile_pool(name="psumP", bufs=2, space="PSUM") as psum:

            # constants / misc, emitted first so they overlap the loads
            ident = post.tile([P, P], F32, tag="ident")
            masks.make_identity(nc, ident[:])
            ones = post.tile([P, 1], F32, tag="ones")
            nc.vector.memset(ones[:], 1.0)
            # iota: value = t+1 = th*128 + p + 1 per column k=(b,th)
            posf1 = post.tile([P, NT], F32, tag="posf1")
            nc.gpsimd.iota(posf1[:], pattern=[[0, KB], [128, TH]], base=1,
                           channel_multiplier=1,
                           allow_small_or_imprecise_dtypes=True)
            # preload the local_scatter ucode library so the reload's engine
            # drain happens before phase 1, not on the tail
            try:
                from concourse import library_config
                nc.gpsimd.load_library(library_config.local_scatter)
            except Exception:
                pass
            zer = post.tile([KB, T], F32, tag="zer")
            nc.vector.memset(zer[:], 0.0)

            # Per-row accumulators, written tile-column-at-a-time.
            macc = acc.tile([P, NT * W8], F32, tag="macc")     # top8 values
            gacc = acc.tile([P, NT * W8], U32, tag="gacc")     # top8 indices
            sacc = acc.tile([P, NT], F32, tag="sacc")          # sum(exp(x))

            for k2 in range(NT // 2):
                xt = xpool.tile([P, 2 * C], F32, tag="xt")
                nc.sync.dma_start(
                    xt[:].rearrange("p (two c) -> p two c", two=2), xv2[k2])
                for two in range(2):
                    k = 2 * k2 + two
                    xh = xt[:, two * C:(two + 1) * C]
                    m8 = macc[:, k * W8:(k + 1) * W8]
                    nc.vector.max(m8, xh)
                    nc.vector.max_index(gacc[:, k * W8:(k + 1) * W8], m8, xh)
                    sc = epool.tile([P, C], BF16, tag="sc")
                    nc.scalar.activation(sc[:], xh, AF.Exp,
                                         accum_out=sacc[:, k:k + 1])

            # ---- epilogue in [P, NT] layout (row-major (b,t) rows) ----
            mv = macc[:].rearrange("p (k e) -> p k e", e=W8)[:, :, 0]   # [P,NT]
            gv = gacc[:].rearrange("p (k e) -> p k e", e=W8)[:, :, 0]

            ls = post.tile([P, NT], F32, tag="ls")
            nc.scalar.activation(ls[:], sacc[:], AF.Ln)
            ct = post.tile([P, NT], F32, tag="ct")    # contrib = m - ln(sumexp)
            nc.vector.tensor_tensor(ct[:], mv, ls[:], OP.subtract)
            gf = post.tile([P, NT], F32, tag="gf")    # argmax ids as f32
            nc.vector.tensor_copy(gf[:], gv)

            # scores[b] = sum_t ct: PE column-sum then group-of-8 reduce
            cps = psum.tile([1, NT], F32, tag="cps")
            nc.tensor.matmul(cps[:], ones[:, 0:1], ct[:])     # [1, NT] colsums
            s8 = post.tile([1, KB], F32, tag="s8")
            nc.vector.tensor_reduce(
                s8[:], cps[:].rearrange("o (b h) -> o b h", b=KB),
                axis=mybir.AxisListType.X, op=OP.add)
            nc.sync.dma_start(scores, s8[:])

            # enc = nonblank ? (t+1) + g/1024 : 0, built in [P, NT] layout.
            # (exact in f32: (1024*(t+1)+g) < 2^21)
            nb128 = post.tile([P, NT], F32, tag="nb128")
            nc.vector.tensor_scalar(nb128[:], gf[:], 0.0, None, OP.not_equal)
            e1 = post.tile([P, NT], F32, tag="e1")
            nc.vector.scalar_tensor_tensor(e1[:], gf[:], 1.0 / 1024.0, posf1[:],
                                           op0=OP.mult, op1=OP.add)
            enc128 = post.tile([P, NT], F32, tag="enc128")
            nc.vector.tensor_tensor(enc128[:], e1[:], nb128[:], OP.mult)

            gb = post.tile([KB, T], F32, tag="gb")
            encb = post.tile([KB, T], F32, tag="encb")
            for src, dst, tg, dmae in ((gf, gb, "tgf", nc.scalar),
                                       (enc128, encb, "tenc", nc.sync)):
                pt = psum.tile([NT, P], F32, tag="pt")
                nc.tensor.transpose(pt[:], src[:], ident[:])
                sb = post.tile([NT, P], F32, tag=tg)
                nc.vector.tensor_copy(sb[:], pt[:])
                for b in range(KB):
                    dmae.dma_start(dst[b:b + 1, :],
                                   sb[b * TH:(b + 1) * TH, :])

            # ---- collapse of the greedy path, [KB, T] layout ----
            efull = post.tile([KB, T + 1], F32, tag="efull")
            nc.vector.memset(efull[:, 0:1], 0.0)
            nc.vector.tensor_tensor_scan(efull[:, 1:T + 1], encb[:], zer[:],
                                         0.0, OP.max, OP.add)
            pe = efull[:, 0:T]                       # enc of last nonblank < t
            # prev char = frac(pe) * 1024; sentinel 0 -> 0 = blank
            ri = post.tile([KB, T], I32, tag="ri")
            nc.vector.tensor_copy(ri[:], pe)         # trunc (pe >= 0)
            fr = post.tile([KB, T], F32, tag="fr")   # fr = pe - trunc(pe)
            nc.vector.scalar_tensor_tensor(fr[:], ri[:], -1.0, pe,
                                           op0=OP.mult, op1=OP.add)
            pc = post.tile([KB, T], F32, tag="pc")   # on ACT: fr*1024
            nc.scalar.activation(pc[:], fr[:], AF.Copy, scale=1024.0)
            ne2 = post.tile([KB, T], F32, tag="ne2")
            nc.vector.tensor_tensor(ne2[:], gb[:], pc[:], OP.not_equal)
            keep = post.tile([KB, T], F32, tag="keep")   # (gb != 0) * ne2
            nc.vector.scalar_tensor_tensor(keep[:], gb[:], 0.0, ne2[:],
                                           op0=OP.not_equal, op1=OP.mult)

            csum = post.tile([KB, T], F32, tag="csum")
            nc.vector.tensor_tensor_scan(csum[:], keep[:], zer[:], 0.0,
                                         OP.add, OP.add)
            leni = post.tile([KB, 1], I32, tag="leni")
            nc.scalar.activation(leni[:], csum[:, T - 1:T], AF.Copy)
            nc.scalar.dma_start(lengths, leni[:])

            sidf = post.tile([KB, T], F32, tag="sidf")
            nc.vector.tensor_tensor(sidf[:], keep[:], csum[:], OP.mult)
            sid16 = post.tile([16, T], I16, tag="sid16")
            nc.vector.memset(sid16[:], -1)
            nc.scalar.activation(sid16[0:KB, :], sidf[:], AF.Copy, bias=-1.0)
            dat16 = post.tile([16, T], I16, tag="dat16")
            nc.vector.memset(dat16[:], 0)
            nc.scalar.activation(dat16[0:KB, :], gb[:], AF.Copy, bias=1.0)
            dec16 = post.tile([16, T], I16, tag="dec16")
            nc.gpsimd.local_scatter(dec16[:], dat16[:], sid16[:], channels=16,
                                    num_elems=T, num_idxs=T)
            dec32 = post.tile([KB, T], I32, tag="dec32")
            nc.scalar.activation(dec32[:], dec16[0:KB, :], AF.Copy, bias=-1.0)
            nc.sync.dma_start(decoded, dec32[:])

    nc.compile()
    return nc


def _get_nc():
    if "nc" not in _CACHE:
        _CACHE["nc"] = _build()
    return _CACHE["nc"]


def kernel(logits: np.ndarray, _trace: bool = False, _result_box: dict | None = None):
    nc = _get_nc()
    logits = np.ascontiguousarray(logits, dtype=np.float32)
    in_maps = [
        {"logits": logits[c * KB:(c + 1) * KB]} for c in range(NCORES)
    ]
    res = run_bass_kernel_spmd(nc, in_maps, core_ids=list(range(NCORES)),
                               trace=_trace)
    if _result_box is not None:
        _result_box["res"] = res
    decoded = np.concatenate([res.results[c]["decoded"] for c in range(NCORES)], axis=0)
    lengths = np.concatenate([res.results[c]["lengths"] for c in range(NCORES)], axis=0)
    scores = np.concatenate([res.results[c]["scores"] for c in range(NCORES)], axis=0)
    return decoded, lengths.reshape(B), scores.reshape(B)


# revision 23
# speedup vs baseline: 1.0147x; 1.0090x over previous
"""CTC beam-search decoder kernel for Trainium2 (8 NeuronCores, data-parallel).

Math note: the reference keeps (prefix, score) beams with NO prefix merging and
expands every beam with the SAME per-step log-prob vector, taking a global
top-W each step.  Under jax.lax.top_k's descending sort, beam 0 after step t is
always (previous beam 0) extended by the per-step argmax class, and its score
is the running sum of per-step max log-probs:

    best_seq[b, t]  = argmax_c logits[b, t, c]          (logp is monotone in logits)
    scores[b, 0]    = sum_t (max_c logits[b,t,c] - logsumexp_c logits[b,t,c])

so the whole scan collapses to a per-(b,t)-row argmax + logsumexp, followed by
the reference's blank/repeat collapse of the greedy path.

Per core (batch shard of 8 rows): 64 tiles of [128 rows, 512 classes] are
processed with DVE max8/max_index + ACT exp(accum) for logsumexp; the epilogue
transposes per-row results into [8, T] layout, runs the collapse (cummax
forward-fill of last non-blank class, cumsum compaction indices, GPSIMD
local_scatter) and writes decoded/lengths/scores.  Tail scalar-affine ops ride
the otherwise-idle ACT engine; fold DMAs split across the two HWDGE queues.
"""
import os
import sys

sys.path.insert(0, "/opt/trn_rl_repo")

# The execute path reaches the NeuronCores through the axon PJRT plugin; a
# CPU-pinned JAX_PLATFORMS (harmless for this module otherwise) would hide
# the devices from bass2jax.  Only fix it if jax hasn't initialized yet.
if os.environ.get("JAX_PLATFORMS") == "cpu" and "jax" not in sys.modules:
    os.environ["JAX_PLATFORMS"] = "axon"

import numpy as np

import concourse.bacc as bacc
import concourse.mybir as mybir
import concourse.tile as tile
from concourse import masks
from concourse.bass_utils import run_bass_kernel_spmd

B, T, C = 64, 1024, 512
NCORES = 8
KB = B // NCORES          # batch rows per core
P = 128                   # SBUF partitions
NT = KB * T // P          # [P, C] tiles per core
TH = NT // KB             # 128-row chunks per batch row (t = th*128 + p)
W8 = 8                    # max8 width

F32 = mybir.dt.float32
BF16 = mybir.dt.bfloat16
I32 = mybir.dt.int32
I16 = mybir.dt.int16
U32 = mybir.dt.uint32
AF = mybir.ActivationFunctionType
OP = mybir.AluOpType

_CACHE = {}


def _build():
    nc = bacc.Bacc(
        "TRN2",
        target_bir_lowering=False,
        debug=False,
        enable_asserts=False,
        num_devices=1,
    )
    logits = nc.dram_tensor("logits", [KB, T, C], F32, kind="ExternalInput").ap()
    decoded = nc.dram_tensor("decoded", [KB, T], I32, kind="ExternalOutput").ap()
    lengths = nc.dram_tensor("lengths", [KB, 1], I32, kind="ExternalOutput").ap()
    scores = nc.dram_tensor("scores", [KB, 1], F32, kind="ExternalOutput").ap()

    # row r = b*T + t = 128*(2*k2 + two) + p ; double-tile loads
    xv2 = logits.rearrange("b t c -> (b t) c").rearrange(
        "(n two p) c -> n p two c", two=2, p=P)

    with tile.TileContext(nc) as tc:
        with tc.tile_pool(name="xin", bufs=6) as xpool, \
             tc.tile_pool(name="expo", bufs=4) as epool, \
             tc.tile_pool(name="acc", bufs=1) as acc, \
             tc.tile_pool(name="post", bufs=1) as post, \
             tc.tile_pool(name="psumP", bufs=2, space="PSUM") as psum:

            # constants / misc, emitted first so they overlap the loads
            ident = post.tile([P, P], F32, tag="ident")
            masks.make_identity(nc, ident[:])
            ones = post.tile([P, 1], F32, tag="ones")
            nc.vector.memset(ones[:], 1.0)
            # iota: value = t+1 = th*128 + p + 1 per column k=(b,th)
            posf1 = post.tile([P, NT], F32, tag="posf1")
            nc.gpsimd.iota(posf1[:], pattern=[[0, KB], [128, TH]], base=1,
                           channel_multiplier=1,
                           allow_small_or_imprecise_dtypes=True)
            # preload the local_scatter ucode library so the reload's engine
            # drain happens before phase 1, not on the tail
            try:
                from concourse import library_config
                nc.gpsimd.load_library(library_config.local_scatter)
            except Exception:
                pass
            zer = post.tile([KB, T], F32, tag="zer")
            nc.vector.memset(zer[:], 0.0)

            # Per-row accumulators, written tile-column-at-a-time.
            macc = acc.tile([P, NT * W8], F32, tag="macc")     # top8 values
            gacc = acc.tile([P, NT * W8], U32, tag="gacc")     # top8 indices
            sacc = acc.tile([P, NT], F32, tag="sacc")          # sum(exp(x))

            for k2 in range(NT // 2):
                xt = xpool.tile([P, 2 * C], F32, tag="xt")
                nc.sync.dma_start(
                    xt[:].rearrange("p (two c) -> p two c", two=2), xv2[k2])
                for two in range(2):
                    k = 2 * k2 + two
                    xh = xt[:, two * C:(two + 1) * C]
                    m8 = macc[:, k * W8:(k + 1) * W8]
                    nc.vector.max(m8, xh)
                    nc.vector.max_index(gacc[:, k * W8:(k + 1) * W8], m8, xh)
                    sc = epool.tile([P, C], BF16, tag="sc")
                    nc.scalar.activation(sc[:], xh, AF.Exp,
                                         accum_out=sacc[:, k:k + 1])

            # ---- epilogue in [P, NT] layout (row-major (b,t) rows) ----
            mv = macc[:].rearrange("p (k e) -> p k e", e=W8)[:, :, 0]   # [P,NT]
            gv = gacc[:].rearrange("p (k e) -> p k e", e=W8)[:, :, 0]

            ls = post.tile([P, NT], F32, tag="ls")
            nc.scalar.activation(ls[:], sacc[:], AF.Ln)
            ct = post.tile([P, NT], F32, tag="ct")    # contrib = m - ln(sumexp)
            nc.vector.tensor_tensor(ct[:], mv, ls[:], OP.subtract)

            # scores[b] = sum_t ct: PE column-sum then group-of-8 reduce
            cps = psum.tile([1, NT], F32, tag="cps")
            nc.tensor.matmul(cps[:], ones[:, 0:1], ct[:])     # [1, NT] colsums
            s8 = post.tile([1, KB], F32, tag="s8")
            nc.vector.tensor_reduce(
                s8[:], cps[:].rearrange("o (b h) -> o b h", b=KB),
                axis=mybir.AxisListType.X, op=OP.add)
            nc.sync.dma_start(scores, s8[:])

            # enc = nonblank ? (t+1) + g/1024 : 0, built in [P, NT] layout.
            # (exact in f32: (1024*(t+1)+g) < 2^21)
            # gf and enc live in one [P, 2*NT] tile -> single PE transpose.
            ge = post.tile([P, 2 * NT], F32, tag="ge")
            gfv = ge[:, 0:NT]
            nc.vector.tensor_copy(gfv, gv)
            nb128 = post.tile([P, NT], F32, tag="nb128")
            nc.vector.tensor_scalar(nb128[:], gfv, 0.0, None, OP.not_equal)
            e1 = post.tile([P, NT], F32, tag="e1")
            nc.vector.scalar_tensor_tensor(e1[:], gfv, 1.0 / 1024.0, posf1[:],
                                           op0=OP.mult, op1=OP.add)
            nc.vector.tensor_tensor(ge[:, NT:2 * NT], e1[:], nb128[:], OP.mult)

            pt = psum.tile([2 * NT, P], F32, tag="pt")
            nc.tensor.transpose(pt[:], ge[:], ident[:])
            sb = post.tile([2 * NT, P], F32, tag="sbT")
            nc.vector.tensor_copy(sb[:], pt[:])
            gb = post.tile([KB, T], F32, tag="gb")
            encb = post.tile([KB, T], F32, tag="encb")
            for b in range(KB):
                nc.scalar.dma_start(gb[b:b + 1, :],
                                    sb[b * TH:(b + 1) * TH, :])
                nc.sync.dma_start(encb[b:b + 1, :],
                                  sb[NT + b * TH:NT + (b + 1) * TH, :])

            # ---- collapse of the greedy path, [KB, T] layout ----
            efull = post.tile([KB, T + 1], F32, tag="efull")
            nc.vector.memset(efull[:, 0:1], 0.0)
            nc.vector.tensor_tensor_scan(efull[:, 1:T + 1], encb[:], zer[:],
                                         0.0, OP.max, OP.add)
            # keep[t] = nonblank[t] & (frac(e[<=t]) != frac(e[<t])): when
            # nonblank, cummax-incl equals enc[t] so its frac is g[t]/1024 and
            # the previous frac is the last non-blank char (0 = blank sentinel)
            ri2 = post.tile([KB, T + 1], I32, tag="ri2")
            nc.vector.tensor_copy(ri2[:], efull[:])  # trunc (e >= 0)
            fr2 = post.tile([KB, T + 1], F32, tag="fr2")
            nc.vector.scalar_tensor_tensor(fr2[:], ri2[:], -1.0, efull[:],
                                           op0=OP.mult, op1=OP.add)
            dfr = post.tile([KB, T], F32, tag="dfr")
            nc.vector.tensor_tensor(dfr[:], fr2[:, 1:T + 1], fr2[:, 0:T],
                                    OP.not_equal)
            keep = post.tile([KB, T], F32, tag="keep")   # (gb != 0) * dfr
            nc.vector.scalar_tensor_tensor(keep[:], gb[:], 0.0, dfr[:],
                                           op0=OP.not_equal, op1=OP.mult)

            csum = post.tile([KB, T], F32, tag="csum")
            nc.vector.tensor_tensor_scan(csum[:], keep[:], zer[:], 0.0,
                                         OP.add, OP.add)
            leni = post.tile([KB, 1], I32, tag="leni")
            nc.scalar.activation(leni[:], csum[:, T - 1:T], AF.Copy)
            nc.scalar.dma_start(lengths, leni[:])

            sidf = post.tile([KB, T], F32, tag="sidf")
            nc.vector.tensor_tensor(sidf[:], keep[:], csum[:], OP.mult)
            sid16 = post.tile([16, T], I16, tag="sid16")
            nc.vector.memset(sid16[:], -1)
            nc.scalar.activation(sid16[0:KB, :], sidf[:], AF.Copy, bias=-1.0)
            dat16 = post.tile([16, T], I16, tag="dat16")
            nc.vector.memset(dat16[:], 0)
            nc.scalar.activation(dat16[0:KB, :], gb[:], AF.Copy, bias=1.0)
            dec16 = post.tile([16, T], I16, tag="dec16")
            nc.gpsimd.local_scatter(dec16[:], dat16[:], sid16[:], channels=16,
                                    num_elems=T, num_idxs=T)
            dec32 = post.tile([KB, T], I32, tag="dec32")
            nc.scalar.activation(dec32[:], dec16[0:KB, :], AF.Copy, bias=-1.0)
            nc.sync.dma_start(decoded, dec32[:])

    nc.compile()
    return nc


def _get_nc():
    if "nc" not in _CACHE:
        _CACHE["nc"] = _build()
    return _CACHE["nc"]


def kernel(logits: np.ndarray, _trace: bool = False, _result_box: dict | None = None):
    nc = _get_nc()
    logits = np.ascontiguousarray(logits, dtype=np.float32)
    in_maps = [
        {"logits": logits[c * KB:(c + 1) * KB]} for c in range(NCORES)
    ]
    res = run_bass_kernel_spmd(nc, in_maps, core_ids=list(range(NCORES)),
                               trace=_trace)
    if _result_box is not None:
        _result_box["res"] = res
    decoded = np.concatenate([res.results[c]["decoded"] for c in range(NCORES)], axis=0)
    lengths = np.concatenate([res.results[c]["lengths"] for c in range(NCORES)], axis=0)
    scores = np.concatenate([res.results[c]["scores"] for c in range(NCORES)], axis=0)
    return decoded, lengths.reshape(B), scores.reshape(B)
